# revision 15
# baseline (speedup 1.0000x reference)
"""Trainium2 Bass kernel for nn_Block_3539053052091 (hedgehog-style linear
attention block with ALiBi-decay mask, smeared keys, and sandwich layernorms).

v2 strategy (8 NeuronCores, heads sharded 2/core):
  - all big matmuls in bf16 (1 PE cycle/row vs 4 for f32).
  - LN folded into weights; mean/std enter as a rank-2 PSUM fixup; the
    per-token rstd rides the activation-scale of the exp/silu ops.
  - q's softmax normalizer and both 1/s scales fold EXACTLY into the
    attention-denominator eps term (per-row invariance), so q needs no
    elementwise normalization at all.
  - all transposes ride the DMA XBAR (dma_start_transpose, bf16) instead
    of PE+PSUM round-trips.
  - elementwise work is split across DVE / Pool / Act so no single engine
    bottlenecks; PE is the roofline.
  - the b h i j attention is chunked linear attention (chunk 128) with a
    decayed running state S (f32 master + bf16 matmul copy).
  - po ships row-major to DRAM; the AllToAll runs in 4 chunks overlapped
    with the main loop, each followed by that chunk's W_out projection +
    final LN (receiver side transposes via DMA from DRAM).
"""

import numpy as np

import concourse.bass as bass
import concourse.mybir as mybir
import concourse.tile as tile
from concourse import bacc

f32 = mybir.dt.float32
bf16 = mybir.dt.bfloat16

N_CORES = 8
B = 2
L = 2048
D_MODEL = 1024
HEADS = 16
EXP = 2
D_EXP = D_MODEL * EXP          # 2048
D_HEAD = D_EXP // HEADS        # 128
HPC = HEADS // N_CORES         # heads per core = 2
C = 128                        # chunk (= row tile) size
ROWS = B * L                   # 4096 flattened rows
NT = ROWS // C                 # 32 row tiles
TPB = L // C                   # 16 tiles per batch
KT = D_MODEL // 128            # 8 contraction tiles
NKT = D_EXP // 128             # 16 contraction tiles for out proj
GROUPS = 4                     # collective chunks
GSZ = NT // GROUPS             # 8 tiles per group (= one dest tile per core)
OUTPROJ_DELAY = 6              # tiles between a collective and its out proj
LN_EPS = 1e-5
ATTN_EPS = 1e-5

Act = mybir.ActivationFunctionType
Alu = mybir.AluOpType

# structural decay constants (depend only on head count, not on inputs)
_H2 = HEADS // 2
_SLOPES = np.concatenate([2.0 ** np.linspace(0.0, -8.0, _H2),
                          np.zeros(HEADS - _H2)]).astype(np.float64)


def build_kernel(mm_dt: str = "bf16", reps: int = 1,
                 trans_mode: str = "dma", silu_mode: str = "manual"):
    del mm_dt  # single bf16 path
    nc = bacc.Bacc("TRN2", target_bir_lowering=False, debug=False,
                   num_devices=N_CORES)

    x_in = nc.dram_tensor("x", [ROWS, D_MODEL], bf16, kind="ExternalInput")
    xt_in = nc.dram_tensor("xt", [D_MODEL, ROWS], bf16, kind="ExternalInput")
    wvp_in = nc.dram_tensor("wvp", [D_MODEL, 4 * D_HEAD], bf16, kind="ExternalInput")
    wq_in = nc.dram_tensor("wq", [D_MODEL, 4 * D_HEAD], bf16, kind="ExternalInput")
    fvp_in = nc.dram_tensor("fvp", [2, 4 * D_HEAD], bf16, kind="ExternalInput")
    fq_in = nc.dram_tensor("fq", [2, 4 * D_HEAD], bf16, kind="ExternalInput")
    wout_in = nc.dram_tensor("wout", [D_EXP, D_MODEL], bf16, kind="ExternalInput")
    outw_in = nc.dram_tensor("outw", [D_MODEL], f32, kind="ExternalInput")
    outb_in = nc.dram_tensor("outb", [D_MODEL], f32, kind="ExternalInput")
    dt_in = nc.dram_tensor("dtmask", [HPC, C, C], f32, kind="ExternalInput")
    # per-token-per-head tiles, token on partitions
    lamf_in = nc.dram_tensor("lamf", [C, HPC, D_HEAD + 1], f32, kind="ExternalInput")
    musf_in = nc.dram_tensor("musf", [C, HPC * D_HEAD], bf16, kind="ExternalInput")
    sigf_in = nc.dram_tensor("sigf", [C, HPC * D_HEAD], bf16, kind="ExternalInput")
    omsigf_in = nc.dram_tensor("omsigf", [C, HPC * D_HEAD], bf16, kind="ExternalInput")
    epss_in = nc.dram_tensor("epss", [C, HPC], f32, kind="ExternalInput")

    RB = ROWS // N_CORES  # 512 output rows per core
    out_ext = nc.dram_tensor("out", [RB, D_MODEL], f32, kind="ExternalOutput")
    nex = 2 if reps > 1 else 1
    pot_dram = nc.dram_tensor("pot", [nex, GROUPS, N_CORES, C, HPC * D_HEAD], bf16)
    potex_dram = nc.dram_tensor("potex", [nex, GROUPS, N_CORES, C, HPC * D_HEAD], bf16)

    gamc = [float(np.exp(-_SLOPES[HPC * 0 + h] * C)) for h in range(HPC)]
    # NOTE: gamc indexed per core at runtime? -- slopes differ per core!
    # gamc must come from a tensor since this SPMD program runs on all cores.
    gamcf_in = nc.dram_tensor("gamcf", [C, HPC], f32, kind="ExternalInput")

    def bcast_ap(handle, parts=128):
        ap = handle.ap()
        return bass.AP(tensor=ap.tensor, offset=ap.offset,
                       ap=[[0, parts]] + list(ap.ap))

    xt_ap = xt_in.ap().rearrange("(kt p) r -> p kt r", p=128)

    with tile.TileContext(nc) as tc:
        with (
            tc.tile_pool(name="const", bufs=1) as cst,
            tc.tile_pool(name="xp", bufs=2) as xp,
            tc.tile_pool(name="work", bufs=2) as wk,
            tc.tile_pool(name="small", bufs=4) as sm,
            tc.tile_pool(name="state", bufs=2) as st,
            tc.tile_pool(name="statp", bufs=1) as sp,
            tc.tile_pool(name="zsb", bufs=2) as zrp,
            tc.tile_pool(name="pproj", bufs=2, space="PSUM") as pproj,
            tc.tile_pool(name="pat", bufs=1, space="PSUM") as pat,
            tc.tile_pool(name="po12", bufs=1, space="PSUM") as pO,
            tc.tile_pool(name="psst", bufs=1, space="PSUM") as pS,
        ):
            # ---- constants ----
            eps_t = cst.tile([128, 1], f32)
            nc.vector.memset(eps_t[:], LN_EPS)

            wvp_sb = cst.tile([128, KT, 4 * D_HEAD], bf16)
            wq_sb = cst.tile([128, KT, 4 * D_HEAD], bf16)
            wout_sb = cst.tile([128, NKT, D_MODEL], bf16)
            for dst, src in ((wvp_sb, wvp_in), (wq_sb, wq_in),
                             (wout_sb, wout_in)):
                nc.sync.dma_start(
                    out=dst, in_=src.ap().rearrange("(kt p) n -> p kt n", p=128))
            fvp_sb = cst.tile([2, 4 * D_HEAD], bf16)
            fq_sb = cst.tile([2, 4 * D_HEAD], bf16)
            nc.sync.dma_start(out=fvp_sb, in_=fvp_in.ap())
            nc.sync.dma_start(out=fq_sb, in_=fq_in.ap())

            dt_sb = cst.tile([128, HPC, C], f32)
            nc.sync.dma_start(out=dt_sb, in_=dt_in.ap().rearrange("h b a -> b h a"))
            lamf = cst.tile([128, HPC, D_HEAD + 1], f32)
            nc.sync.dma_start(out=lamf, in_=lamf_in.ap())
            musf = cst.tile([128, HPC * D_HEAD], bf16)
            sigf = cst.tile([128, HPC * D_HEAD], bf16)
            omsigf = cst.tile([128, HPC * D_HEAD], bf16)
            for dst, src in ((musf, musf_in), (sigf, sigf_in), (omsigf, omsigf_in)):
                nc.sync.dma_start(out=dst, in_=src.ap())
            epss_t = cst.tile([128, HPC], f32)
            nc.sync.dma_start(out=epss_t, in_=epss_in.ap())
            gamcf = cst.tile([128, HPC], f32)
            nc.sync.dma_start(out=gamcf, in_=gamcf_in.ap())

            outw_bc = cst.tile([128, D_MODEL], f32)
            outb_bc = cst.tile([128, D_MODEL], f32)
            nc.sync.dma_start(out=outw_bc, in_=bcast_ap(outw_in))
            nc.sync.dma_start(out=outb_bc, in_=bcast_ap(outb_in))

            ident = None
            if trans_mode == "pe":
                from concourse.masks import make_identity
                ident = cst.tile([128, 128], bf16)
                make_identity(nc, ident[:])

            def transpose128(out_sl, in_sl, tag):
                """out_sl [128,128] bf16 <- transpose of in_sl [128,128] bf16."""
                if trans_mode == "dma":
                    nc.sync.dma_start_transpose(out=out_sl, in_=in_sl)
                else:
                    tp = pat.tile([128, 128], bf16, tag="tp", name=f"tp_{tag}")
                    nc.tensor.matmul(tp[:], in_sl, ident[:], is_transpose=True)
                    nc.vector.tensor_copy(out=out_sl, in_=tp[:])

            def emit_outproj(rep, g):
                pex = potex_dram[rep % nex, g]
                pox = wk.tile([128, NKT, C], bf16, tag="pox",
                              name=f"pox{rep}_{g}")
                for s in range(N_CORES):
                    for ci in range(HPC):
                        nc.scalar.dma_start_transpose(
                            out=pox[:, s * HPC + ci, :],
                            in_=pex[s, :, ci * 128:(ci + 1) * 128])
                z_sb = zrp.tile([128, D_MODEL], f32, tag="zsb")
                statf = sm.tile([128, 2, 6], f32, tag="statf")
                for half in range(2):
                    ns = slice(half * 512, (half + 1) * 512)
                    z_ps = pproj.tile([128, 512], f32, tag="pvp",
                                      name=f"z{rep}_{g}_{half}")
                    for kt in range(NKT):
                        nc.tensor.matmul(z_ps[:], pox[:, kt, :],
                                         wout_sb[:, kt, ns],
                                         start=(kt == 0),
                                         stop=(kt == NKT - 1))
                    nc.vector.tensor_copy(out=z_sb[:, ns], in_=z_ps[:])
                    nc.vector.bn_stats(out=statf[:, half, :],
                                       in_=z_sb[:, ns])
                mvf = sm.tile([128, 2], f32, tag="mvf")
                nc.vector.bn_aggr(out=mvf[:], in_=statf[:])
                lnf = sm.tile([128, 1], f32, tag="lnf")
                nc.scalar.activation(out=lnf[:], in_=mvf[:, 1:2],
                                     func=Act.Ln, bias=eps_t[:])
                rstdf = sm.tile([128, 1], f32, tag="rstdf")
                nc.scalar.activation(out=rstdf[:], in_=lnf[:],
                                     func=Act.Exp, scale=-0.5)
                o_t = xp.tile([128, D_MODEL], f32, tag="y")
                nc.vector.tensor_scalar(
                    out=o_t[:], in0=z_sb[:], scalar1=mvf[:, 0:1],
                    scalar2=rstdf[:], op0=Alu.subtract, op1=Alu.mult)
                nc.gpsimd.tensor_tensor(out=o_t[:], in0=o_t[:],
                                        in1=outw_bc[:], op=Alu.mult)
                nc.gpsimd.tensor_tensor(out=o_t[:], in0=o_t[:],
                                        in1=outb_bc[:], op=Alu.add)
                nc.sync.dma_start(out=out_ext[g * C:(g + 1) * C, :],
                                  in_=o_t[:])

            for rep in range(reps):
                emitted = set()
                # ---- LN stats prepass (Pool engine) ----
                mv_all = sp.tile([128, NT, 2], f32, tag="mv_all")
                for t in range(NT):
                    x_t = xp.tile([128, D_MODEL], bf16, tag="x")
                    nc.sync.dma_start(out=x_t, in_=x_in[t * C:(t + 1) * C, :])
                    stats = sm.tile([128, 2, 6], f32, tag="stats")
                    for i in range(2):
                        nc.vector.bn_stats(out=stats[:, i, :],
                                           in_=x_t[:, i * 512:(i + 1) * 512])
                    nc.vector.bn_aggr(out=mv_all[:, t, :], in_=stats[:])
                ln_all = sp.tile([128, NT], f32, tag="ln_all")
                rstd_all = sp.tile([128, NT], f32, tag="rstd_all")
                nc.scalar.activation(out=ln_all[:], in_=mv_all[:, :, 1],
                                     func=Act.Ln, bias=eps_t[:])
                nc.scalar.activation(out=rstd_all[:], in_=ln_all[:],
                                     func=Act.Exp, scale=-0.5)
                msn_bf = sp.tile([128, 128], bf16, tag="msn_bf")
                nc.gpsimd.memset(msn_bf[:], 0.0)
                msn_v = msn_bf[:, 0:2 * NT].rearrange("p (t s) -> p t s", s=2)
                nc.vector.tensor_scalar_mul(out=msn_v[:, :, 0:1],
                                            in0=mv_all[:, :, 0:1], scalar1=-1.0)
                nc.scalar.activation(
                    out=msn_v[:, :, 1:2],
                    in_=ln_all[:].rearrange("p (t o) -> p t o", o=1),
                    func=Act.Exp, scale=0.5)
                mustd_all = sp.tile([128, 128], bf16, tag="mustd_all")
                if trans_mode == "dma":
                    nc.sync.dma_start_transpose(out=mustd_all[:], in_=msn_bf[:])
                else:
                    tpm = pat.tile([128, 128], bf16, tag="tp", name="tp_msn")
                    nc.tensor.matmul(tpm[:], msn_bf[:], ident[:], is_transpose=True)
                    nc.vector.tensor_copy(out=mustd_all[:], in_=tpm[:])
                # matmul lhsT must sit at base partition 0: restage the
                # transposed stat pairs into a flat [2, ROWS] tile
                mustd_flat = sp.tile([2, ROWS], bf16, tag="mustd_flat")
                for t in range(NT):
                    nc.scalar.dma_start(out=mustd_flat[:, t * C:(t + 1) * C],
                                        in_=mustd_all[2 * t:2 * t + 2, :])

                S_f = None
                S_bf = None
                carry = None
                for t in range(NT):
                    chunk = t % TPB
                    if chunk == 0:
                        S_f = st.tile([128, HPC, D_HEAD + 1], f32, tag="Sf",
                                      name=f"Sf_init{t}")
                        nc.vector.memset(S_f[:], 0.0)
                        S_bf = st.tile([128, HPC, D_HEAD + 1], bf16, tag="Sbf",
                                       name=f"Sbf_init{t}")
                        nc.gpsimd.memset(S_bf[:], 0.0)
                        carry = st.tile([1, 2 * D_HEAD], bf16, tag="carry",
                                        name=f"carry_init{t}")
                        nc.gpsimd.memset(carry[:], 0.0)
                    rstd = rstd_all[:, t:t + 1]

                    # ---- projections ----
                    xT = wk.tile([128, KT, C], bf16, tag="xT")
                    nc.sync.dma_start(out=xT, in_=xt_ap[:, :, t * C:(t + 1) * C])
                    ps_vp = pproj.tile([128, 4 * D_HEAD], f32, tag="pvp")
                    ps_qk = pproj.tile([128, 4 * D_HEAD], f32, tag="pqk")
                    for k in range(KT):
                        nc.tensor.matmul(ps_vp[:], xT[:, k, :], wvp_sb[:, k, :],
                                         start=(k == 0), stop=False)
                        nc.tensor.matmul(ps_qk[:], xT[:, k, :], wq_sb[:, k, :],
                                         start=(k == 0), stop=False)
                    mustd = mustd_flat[:, t * C:(t + 1) * C]
                    nc.tensor.matmul(ps_vp[:], mustd, fvp_sb[:],
                                     start=False, stop=True)
                    nc.tensor.matmul(ps_qk[:], mustd, fq_sb[:],
                                     start=False, stop=True)

                    # ---- feature maps (Act engine) ----
                    if silu_mode == "act":
                        silu_p = wk.tile([128, 2 * D_HEAD], f32, tag="silup")
                        nc.scalar.activation(out=silu_p[:],
                                             in_=ps_vp[:, 2 * D_HEAD:4 * D_HEAD],
                                             func=Act.Silu, scale=rstd)
                    elif silu_mode == "manual":
                        sig_p = wk.tile([128, 2 * D_HEAD], f32, tag="sigp")
                        nc.scalar.activation(out=sig_p[:],
                                             in_=ps_vp[:, 2 * D_HEAD:4 * D_HEAD],
                                             func=Act.Sigmoid, scale=rstd)
                        silu_p = wk.tile([128, 2 * D_HEAD], f32, tag="silup")
                        nc.vector.scalar_tensor_tensor(
                            out=silu_p[:], in0=sig_p[:], scalar=rstd,
                            in1=ps_vp[:, 2 * D_HEAD:4 * D_HEAD],
                            op0=Alu.mult, op1=Alu.mult)
                    else:
                        # exp-only route keeps every Act func in one
                        # activation table (no 1.3us table reloads):
                        # silu(p) = rstd*ps_p / (1 + exp(-rstd*ps_p))
                        nrstd = sm.tile([128, 1], f32, tag="nrstd")
                        nc.vector.tensor_scalar_mul(out=nrstd[:], in0=rstd,
                                                    scalar1=-1.0)
                        emp = wk.tile([128, 2 * D_HEAD], f32, tag="emp")
                        nc.scalar.activation(out=emp[:],
                                             in_=ps_vp[:, 2 * D_HEAD:4 * D_HEAD],
                                             func=Act.Exp, scale=nrstd[:])
                        nc.gpsimd.tensor_tensor(out=emp[:], in0=emp[:],
                                                in1=ones_bc[:], op=Alu.add)
                        rsig = wk.tile([128, 2 * D_HEAD], f32, tag="rsig")
                        nc.vector.reciprocal(out=rsig[:], in_=emp[:])
                        silu_p = wk.tile([128, 2 * D_HEAD], f32, tag="silup")
                        nc.vector.scalar_tensor_tensor(
                            out=silu_p[:], in0=rsig[:], scalar=rstd,
                            in1=ps_vp[:, 2 * D_HEAD:4 * D_HEAD],
                            op0=Alu.mult, op1=Alu.mult)
                    expq = wk.tile([128, 2 * D_HEAD], bf16, tag="expq")
                    zq = sm.tile([128, HPC], f32, tag="zq")
                    expk = wk.tile([128, 2 * D_HEAD], f32, tag="expk")
                    zk = sm.tile([128, HPC], f32, tag="zk")
                    for h in range(HPC):
                        hs = slice(h * D_HEAD, (h + 1) * D_HEAD)
                        ks = slice(2 * D_HEAD + h * D_HEAD,
                                   2 * D_HEAD + (h + 1) * D_HEAD)
                        nc.scalar.activation(out=expq[:, hs], in_=ps_qk[:, hs],
                                             func=Act.Exp, scale=rstd,
                                             accum_out=zq[:, h:h + 1])
                        nc.scalar.activation(out=expk[:, hs], in_=ps_qk[:, ks],
                                             func=Act.Exp, scale=rstd,
                                             accum_out=zk[:, h:h + 1])

                    # ---- v_aug ----
                    v_aug = wk.tile([128, HPC, D_HEAD + 1], bf16, tag="vaug")
                    nc.vector.tensor_scalar_mul(
                        out=v_aug[:, :, 0:D_HEAD],
                        in0=ps_vp[:, 0:2 * D_HEAD].rearrange(
                            "p (h d) -> p h d", h=HPC),
                        scalar1=rstd)
                    nc.vector.memset(v_aug[:, :, D_HEAD:D_HEAD + 1], 1.0)

                    # ---- k softmax normalize (Act) ----
                    rzk = sm.tile([128, HPC], f32, tag="rzk")
                    nc.vector.reciprocal(out=rzk[:], in_=zk[:])
                    kn = wk.tile([128, 2 * D_HEAD], bf16, tag="kn")
                    for h in range(HPC):
                        hs = slice(h * D_HEAD, (h + 1) * D_HEAD)
                        nc.scalar.activation(out=kn[:, hs], in_=expk[:, hs],
                                             func=Act.Copy,
                                             scale=rzk[:, h:h + 1])

                    # ---- smear (Pool) ----
                    kprev = wk.tile([128, 2 * D_HEAD], bf16, tag="kprev")
                    nc.sync.dma_start(out=kprev[0:1, :], in_=carry[0:1, :])
                    nc.sync.dma_start(out=kprev[1:128, :], in_=kn[0:127, :])
                    carry_new = st.tile([1, 2 * D_HEAD], bf16, tag="carry",
                                        name=f"carry{t}")
                    nc.sync.dma_start(out=carry_new[:], in_=kn[127:128, :])
                    carry = carry_new
                    ktil = wk.tile([128, 2 * D_HEAD], bf16, tag="ktil")
                    nc.gpsimd.tensor_tensor(out=ktil[:], in0=kn[:], in1=omsigf[:],
                                            op=Alu.mult)
                    kp2 = wk.tile([128, 2 * D_HEAD], bf16, tag="kp2")
                    nc.gpsimd.tensor_tensor(out=kp2[:], in0=kprev[:], in1=sigf[:],
                                            op=Alu.mult)
                    nc.gpsimd.tensor_tensor(out=ktil[:], in0=ktil[:], in1=kp2[:],
                                            op=Alu.add)
                    kmu = wk.tile([128, 2 * D_HEAD], bf16, tag="kmu")
                    nc.gpsimd.tensor_tensor(out=kmu[:], in0=ktil[:], in1=musf[:],
                                            op=Alu.mult)

                    # ---- transposes ----
                    qT = wk.tile([128, HPC, C], bf16, tag="qT")
                    kT = wk.tile([128, HPC, C], bf16, tag="kT")
                    for h in range(HPC):
                        hs = slice(h * D_HEAD, (h + 1) * D_HEAD)
                        transpose128(qT[:, h, :], expq[:, hs], f"q{t}_{h}")
                        transpose128(kT[:, h, :], ktil[:, hs], f"k{t}_{h}")

                    # ---- attention ----
                    at_ps = pat.tile([128, HPC, C], f32, tag="at")
                    for h in range(HPC):
                        nc.tensor.matmul(at_ps[:, h, :], kT[:, h, :], qT[:, h, :],
                                         start=True, stop=True)
                    atm = wk.tile([128, HPC, C], bf16, tag="atm")
                    nc.vector.tensor_tensor(out=atm[:], in0=at_ps[:], in1=dt_sb[:],
                                            op=Alu.mult)
                    o1_ps = pO.tile([128, HPC, D_HEAD + 1], f32, tag="o1")
                    o2_ps = pO.tile([128, HPC, D_HEAD + 1], f32, tag="o2")
                    s_ps = pS.tile([128, HPC, D_HEAD + 1], f32, tag="sps")
                    for h in range(HPC):
                        hs = slice(h * D_HEAD, (h + 1) * D_HEAD)
                        nc.tensor.matmul(o1_ps[:, h, :], atm[:, h, :],
                                         v_aug[:, h, :], start=True, stop=True)
                        nc.tensor.matmul(o2_ps[:, h, :], qT[:, h, :],
                                         S_bf[:, h, :], start=True, stop=True)
                        nc.tensor.matmul(s_ps[:, h, :], kmu[:, hs],
                                         v_aug[:, h, :], start=True, stop=True)

                    o_sb = wk.tile([128, HPC, D_HEAD + 1], f32, tag="osb")
                    nc.vector.tensor_tensor(out=o_sb[:], in0=o2_ps[:], in1=lamf[:],
                                            op=Alu.mult)
                    nc.vector.tensor_tensor(out=o_sb[:], in0=o_sb[:], in1=o1_ps[:],
                                            op=Alu.add)
                    den = sm.tile([128, HPC], f32, tag="den")
                    nc.vector.tensor_tensor(out=den[:], in0=zq[:], in1=epss_t[:],
                                            op=Alu.mult)
                    nc.vector.tensor_tensor(
                        out=den[:], in0=den[:],
                        in1=o_sb[:, :, D_HEAD:D_HEAD + 1].rearrange(
                            "p h o -> p (h o)"),
                        op=Alu.add)
                    rden = sm.tile([128, HPC], f32, tag="rden")
                    nc.vector.reciprocal(out=rden[:], in_=den[:])
                    po = wk.tile([128, 2 * D_HEAD], bf16, tag="po")
                    for h in range(HPC):
                        hs = slice(h * D_HEAD, (h + 1) * D_HEAD)
                        nc.vector.scalar_tensor_tensor(
                            out=po[:, hs], in0=o_sb[:, h, 0:D_HEAD],
                            scalar=rden[:, h:h + 1], in1=silu_p[:, hs],
                            op0=Alu.mult, op1=Alu.mult)

                    # ---- state update ----
                    S_f_new = st.tile([128, HPC, D_HEAD + 1], f32, tag="Sf",
                                      name=f"Sf{t}")
                    for h in range(HPC):
                        nc.vector.scalar_tensor_tensor(
                            out=S_f_new[:, h, :], in0=S_f[:, h, :],
                            scalar=gamcf[:, h:h + 1], in1=s_ps[:, h, :],
                            op0=Alu.mult, op1=Alu.add)
                    S_f = S_f_new
                    S_bf_new = st.tile([128, HPC, D_HEAD + 1], bf16, tag="Sbf",
                                       name=f"Sbf{t}")
                    nc.gpsimd.tensor_copy(out=S_bf_new[:], in_=S_f[:])
                    S_bf = S_bf_new

                    # ---- ship po (row-major) ----
                    g, dest = t // GSZ, t % GSZ
                    nc.scalar.dma_start(out=pot_dram[rep % nex, g, dest],
                                        in_=po[:])

                    # ---- chunked exchange (out-proj deferred) ----
                    if dest == GSZ - 1:
                        pin = pot_dram[rep % nex, g]
                        pex = potex_dram[rep % nex, g]
                        nc.gpsimd.collective_compute(
                            "AllToAll", Alu.bypass,
                            replica_groups=[list(range(N_CORES))],
                            ins=[pin], outs=[pex])
                    # emit group g's out projection DELAY tiles after its
                    # collective was issued, so engines never stall on it
                    if t >= GSZ - 1 + OUTPROJ_DELAY and \
                            (t - OUTPROJ_DELAY) % GSZ == GSZ - 1:
                        gd = (t - OUTPROJ_DELAY) // GSZ
                        emit_outproj(rep, gd)
                        emitted.add(gd)
                for g in range(GROUPS):
                    if g not in emitted:
                        emit_outproj(rep, g)

    nc.compile()
    return nc


def prepare_in_maps(inputs: dict):
    """Host-side: fold LN affines into weights, slice per core, build
    per-head decay/smear constants, cast to bf16."""
    import ml_dtypes
    bf = ml_dtypes.bfloat16

    x = np.ascontiguousarray(np.asarray(inputs["x"], np.float32)
                             .reshape(ROWS, D_MODEL)).astype(bf)
    xt = np.ascontiguousarray(x.T)
    W_in = np.asarray(inputs["W_in"], np.float32)
    W_out = np.asarray(inputs["W_out"], np.float32)
    Wq = np.asarray(inputs["Wq"], np.float32)
    Wk = np.asarray(inputs["Wk"], np.float32)
    bq = np.asarray(inputs["bq"], np.float32)
    bk = np.asarray(inputs["bk"], np.float32)
    in_w = np.asarray(inputs["in_ln_w"], np.float32)
    in_b = np.asarray(inputs["in_ln_b"], np.float32)
    q_w = np.asarray(inputs["q_ln_w"], np.float32)
    q_b = np.asarray(inputs["q_ln_b"], np.float32)
    k_w = np.asarray(inputs["k_ln_w"], np.float32)
    k_b = np.asarray(inputs["k_ln_b"], np.float32)
    outw = np.asarray(inputs["out_ln_w"], np.float32)
    outb = np.asarray(inputs["out_ln_b"], np.float32)
    smear = np.asarray(inputs["smear_factor"], np.float32)
    log_scale = np.asarray(inputs["log_scale"], np.float32)

    Wvp_f = W_in * in_w[:, None]
    bvp_f = in_b @ W_in
    Wq_f = Wq * q_w[:, None]
    bq_f = bq + q_b @ Wq
    Wk_f = Wk * k_w[:, None]
    bk_f = bk + k_b @ Wk

    sigm = 1.0 / (1.0 + np.exp(-smear.astype(np.float64)))
    s = np.exp(log_scale.astype(np.float64))

    a = np.arange(C)
    diff = a[:, None] - a[None, :]          # i - j
    ones_col = np.ones((C, 1), np.float32)
    in_maps = []
    for c in range(N_CORES):
        heads = [HPC * c + i for i in range(HPC)]
        vcols = np.concatenate(
            [np.arange(h * D_HEAD, (h + 1) * D_HEAD) for h in heads])
        pcols = vcols + D_EXP
        dts, lams, muss, sigs, omsigs, epsss, gamcs = [], [], [], [], [], [], []
        for h in heads:
            lg = -_SLOPES[h]                 # log gamma
            D = np.where(diff >= 0, np.exp(lg * diff), 0.0)   # [i, j]
            dts.append(D.T.astype(np.float32))                # [j, i]
            lams.append(np.repeat(
                np.exp(lg * (a + 1))[:, None], D_HEAD + 1, 1).astype(np.float32))
            muss.append(np.repeat(
                np.exp(lg * (C - 1 - a))[:, None], D_HEAD, 1).astype(np.float32))
            sigs.append(np.full((C, D_HEAD), sigm[h], np.float32))
            omsigs.append(np.full((C, D_HEAD), 1.0 - sigm[h], np.float32))
            epsss.append(np.full(C, ATTN_EPS * s[h] * s[h], np.float32))
            gamcs.append(np.full(C, np.exp(lg * C), np.float32))
        wvp_c = np.ascontiguousarray(
            np.concatenate([Wvp_f[:, vcols], Wvp_f[:, pcols]], axis=1))
        bvp_c = np.concatenate([bvp_f[vcols], bvp_f[pcols]])
        wq_c = Wq_f[:, vcols]
        wk_c = Wk_f[:, vcols]
        in_maps.append({
            "x": x,
            "xt": xt,
            "wvp": wvp_c.astype(bf),
            "fvp": np.ascontiguousarray(
                np.stack([wvp_c.sum(0), bvp_c])).astype(bf),
            "wq": np.ascontiguousarray(
                np.concatenate([wq_c, wk_c], axis=1)).astype(bf),
            "fq": np.ascontiguousarray(np.stack([
                np.concatenate([wq_c.sum(0), wk_c.sum(0)]),
                np.concatenate([bq_f[vcols], bk_f[vcols]])])).astype(bf),
            "wout": W_out.astype(bf),
            "outw": outw, "outb": outb,
            "dtmask": np.stack(dts),
            "lamf": np.ascontiguousarray(
                np.stack(lams, axis=1)),                      # [C, HPC, 129]
            "musf": np.ascontiguousarray(
                np.concatenate(muss, axis=1)).astype(bf),     # [C, 256]
            "sigf": np.ascontiguousarray(
                np.concatenate(sigs, axis=1)).astype(bf),
            "omsigf": np.ascontiguousarray(
                np.concatenate(omsigs, axis=1)).astype(bf),
            "epss": np.ascontiguousarray(np.stack(epsss, axis=1)),  # [C, HPC]
            "gamcf": np.ascontiguousarray(np.stack(gamcs, axis=1)),
        })
    return in_maps


def assemble_output(outs: list) -> np.ndarray:
    """outs[c] is [512, 1024]: groups g=0..3 rows [g*128:(g+1)*128] hold
    global row tile (g*8 + c)."""
    full = np.empty((ROWS, D_MODEL), np.float32)
    for ti in range(NT):
        c, g = ti % GSZ, ti // GSZ
        full[ti * C:(ti + 1) * C] = outs[c][g * C:(g + 1) * C]
    return full.reshape(B, L, D_MODEL)


DEFAULT_MM_DT = "bf16"

_CACHED = {}


def _get_runner(mm_dt=None, reps=1):
    key = (mm_dt, reps)
    if key not in _CACHED:
        nc = build_kernel(reps=reps)
        _CACHED[key] = nc
    return _CACHED[key]


def kernel(**inputs) -> np.ndarray:
    nc = _get_runner()
    in_maps = prepare_in_maps(inputs)
    from concourse.bass_utils import run_bass_kernel_spmd
    res = run_bass_kernel_spmd(nc, in_maps, list(range(N_CORES)))
    return assemble_output([res.results[c]["out"] for c in range(N_CORES)])


# revision 21
# speedup vs baseline: 1.0413x; 1.0413x over previous
"""Trainium2 Bass kernel for nn_Block_3539053052091 (hedgehog-style linear
attention block with ALiBi-decay mask, smeared keys, and sandwich layernorms).

v3 strategy (8 NeuronCores, heads sharded 2/core):
  - all matmuls bf16 (1 PE cycle/row); LN folded into weights with a
    rank-2 PSUM fixup; per-token rstd rides activation scales.
  - q's softmax normalizer and both 1/s scales fold exactly into the
    attention-denominator eps term, so q needs no elementwise scaling.
  - only Exp/Ln/Copy activations are used -> a single activation table,
    no 1.28us table reloads.
  - per-tile transposes run on the PE (bf16, 53ns) instead of DMA, so no
    DMA latency sits on the per-tile critical path; the smear runs in the
    transposed domain (carry column is an SBUF slice, not a DMA).
  - emission is software-pipelined: tile t+1's projection matmuls are
    emitted before tile t's attention matmuls, so the PE never waits on
    the Act/DVE feature-map chain.
  - PSUM is hand-packed into 8 banks (proj 2x2, transposes, at+o2,
    o1+o2d, s_ps).
  - po ships row-major to DRAM; the AllToAll runs in 4 chunks overlapped
    with the main loop; each chunk's W_out projection + final LN is
    emitted a few tiles later (receiver transposes via DMA from DRAM).
"""

import numpy as np

import concourse.bass as bass
import concourse.mybir as mybir
import concourse.tile as tile
from concourse import bacc
from concourse.masks import make_identity

f32 = mybir.dt.float32
bf16 = mybir.dt.bfloat16

N_CORES = 8
B = 2
L = 2048
D_MODEL = 1024
HEADS = 16
EXP = 2
D_EXP = D_MODEL * EXP          # 2048
D_HEAD = D_EXP // HEADS        # 128
HPC = HEADS // N_CORES         # heads per core = 2
C = 128                        # chunk (= row tile) size
ROWS = B * L                   # 4096 flattened rows
NT = ROWS // C                 # 32 row tiles
TPB = L // C                   # 16 tiles per batch
KT = D_MODEL // 128            # 8 contraction tiles
NKT = D_EXP // 128             # 16 contraction tiles for out proj
GROUPS = 4                     # collective chunks
GSZ = NT // GROUPS             # 8 tiles per group (= one dest tile per core)
OUTPROJ_DELAY = 6              # tiles between a collective and its out proj
LN_EPS = 1e-5
ATTN_EPS = 1e-5

Act = mybir.ActivationFunctionType
Alu = mybir.AluOpType

# structural decay constants (depend only on head count, not on inputs)
_H2 = HEADS // 2
_SLOPES = np.concatenate([2.0 ** np.linspace(0.0, -8.0, _H2),
                          np.zeros(HEADS - _H2)]).astype(np.float64)


def build_kernel(mm_dt: str = "bf16", reps: int = 1, debug_dump=False):
    # debug_dump: False | True (all) | set of dump names
    del mm_dt  # single bf16 path
    if debug_dump is True:
        debug_dump = {"mustd", "expq", "kn", "ktT", "po", "osb"}
    elif not debug_dump:
        debug_dump = set()
    nc = bacc.Bacc("TRN2", target_bir_lowering=False, debug=False,
                   num_devices=N_CORES)

    x_in = nc.dram_tensor("x", [ROWS, D_MODEL], bf16, kind="ExternalInput")
    # tile-contiguous transposed x: [t, p, kt*128+r] = x[t*128+r, kt*128+p]
    xt_in = nc.dram_tensor("xt", [NT, 128, D_MODEL], bf16, kind="ExternalInput")
    wvp_in = nc.dram_tensor("wvp", [D_MODEL, 4 * D_HEAD], bf16, kind="ExternalInput")
    wq_in = nc.dram_tensor("wq", [D_MODEL, 4 * D_HEAD], bf16, kind="ExternalInput")
    fvp_in = nc.dram_tensor("fvp", [2, 4 * D_HEAD], bf16, kind="ExternalInput")
    fq_in = nc.dram_tensor("fq", [2, 4 * D_HEAD], bf16, kind="ExternalInput")
    wout_in = nc.dram_tensor("wout", [D_EXP, D_MODEL], bf16, kind="ExternalInput")
    outw_in = nc.dram_tensor("outw", [D_MODEL], f32, kind="ExternalInput")
    outb_in = nc.dram_tensor("outb", [D_MODEL], f32, kind="ExternalInput")
    dt_in = nc.dram_tensor("dtmask", [HPC, C, C], f32, kind="ExternalInput")
    lamf_in = nc.dram_tensor("lamf", [C, HPC, D_HEAD], f32, kind="ExternalInput")
    lam2_in = nc.dram_tensor("lam2", [C, HPC], f32, kind="ExternalInput")
    musf_in = nc.dram_tensor("musf", [C, HPC * D_HEAD], bf16, kind="ExternalInput")
    sigf_in = nc.dram_tensor("sigf", [C, HPC * D_HEAD], bf16, kind="ExternalInput")
    omsigf_in = nc.dram_tensor("omsigf", [C, HPC * D_HEAD], bf16, kind="ExternalInput")
    epss_in = nc.dram_tensor("epss", [C, HPC], f32, kind="ExternalInput")
    gamcf_in = nc.dram_tensor("gamcf", [C, HPC], f32, kind="ExternalInput")

    RB = ROWS // N_CORES  # 512 output rows per core
    out_ext = nc.dram_tensor("out", [RB, D_MODEL], f32, kind="ExternalOutput")
    dbg = {}
    if "mustd" in debug_dump:
        dbg["mustd"] = nc.dram_tensor("dbg_mustd", [GROUPS, 2, GSZ * C], bf16,
                                      kind="ExternalOutput")
    if "expq" in debug_dump:
        dbg["expq"] = nc.dram_tensor("dbg_expq", [NT, 128, 256], bf16,
                                     kind="ExternalOutput")
    if "kn" in debug_dump:
        dbg["kn"] = nc.dram_tensor("dbg_kn", [NT, 128, 256], bf16,
                                   kind="ExternalOutput")
    if "ktT" in debug_dump:
        dbg["ktT"] = nc.dram_tensor("dbg_ktT", [NT, 128, 256], bf16,
                                    kind="ExternalOutput")
    if "po" in debug_dump:
        dbg["po"] = nc.dram_tensor("dbg_po", [NT, 128, 256], bf16,
                                   kind="ExternalOutput")
    if "osb" in debug_dump:
        dbg["osb"] = nc.dram_tensor("dbg_osb", [NT, 128, 256], f32,
                                    kind="ExternalOutput")
    nex = 2 if reps > 1 else 1
    pot_dram = nc.dram_tensor("pot", [nex, GROUPS, N_CORES, C, HPC * D_HEAD], bf16)
    potex_dram = nc.dram_tensor("potex", [nex, GROUPS, N_CORES, C, HPC * D_HEAD], bf16)

    def bcast_ap(handle, parts=128):
        ap = handle.ap()
        return bass.AP(tensor=ap.tensor, offset=ap.offset,
                       ap=[[0, parts]] + list(ap.ap))

    with tile.TileContext(nc) as tc:
        with (
            tc.tile_pool(name="const", bufs=1) as cst,
            tc.tile_pool(name="xp", bufs=3) as xp,
            tc.tile_pool(name="work", bufs=2) as wk,
            tc.tile_pool(name="small", bufs=4) as sm,
            tc.tile_pool(name="state", bufs=2) as st,
            tc.tile_pool(name="statp", bufs=2) as sp,
            tc.tile_pool(name="zsb", bufs=2) as zrp,
            tc.tile_pool(name="pproj", bufs=2, space="PSUM") as pproj,
            tc.tile_pool(name="ptp", bufs=1, space="PSUM") as ptp,
            tc.tile_pool(name="pato", bufs=1, space="PSUM") as pato,
            tc.tile_pool(name="po1", bufs=1, space="PSUM") as pO1,
            tc.tile_pool(name="psst", bufs=1, space="PSUM") as pS,
        ):
            # ---- constants ----
            eps_t = cst.tile([128, 1], f32)
            nc.vector.memset(eps_t[:], LN_EPS)
            ident = cst.tile([128, 128], bf16)
            make_identity(nc, ident[:])

            wvp_sb = cst.tile([128, KT, 4 * D_HEAD], bf16)
            wq_sb = cst.tile([128, KT, 4 * D_HEAD], bf16)
            wout_sb = cst.tile([128, NKT, D_MODEL], bf16)
            for dst, src in ((wvp_sb, wvp_in), (wq_sb, wq_in),
                             (wout_sb, wout_in)):
                nc.sync.dma_start(
                    out=dst, in_=src.ap().rearrange("(kt p) n -> p kt n", p=128))
            fvp_sb = cst.tile([2, 4 * D_HEAD], bf16)
            fq_sb = cst.tile([2, 4 * D_HEAD], bf16)
            nc.sync.dma_start(out=fvp_sb, in_=fvp_in.ap())
            nc.sync.dma_start(out=fq_sb, in_=fq_in.ap())

            dt_sb = cst.tile([128, HPC, C], f32)
            nc.sync.dma_start(out=dt_sb, in_=dt_in.ap().rearrange("h b a -> b h a"))
            lamf = cst.tile([128, HPC, D_HEAD], f32)
            nc.sync.dma_start(out=lamf, in_=lamf_in.ap())
            lam2 = cst.tile([128, HPC], f32)
            nc.sync.dma_start(out=lam2, in_=lam2_in.ap())
            musf = cst.tile([128, HPC * D_HEAD], bf16)
            sigf = cst.tile([128, HPC * D_HEAD], bf16)
            omsigf = cst.tile([128, HPC * D_HEAD], bf16)
            for dst, src in ((musf, musf_in), (sigf, sigf_in), (omsigf, omsigf_in)):
                nc.sync.dma_start(out=dst, in_=src.ap())
            epss_t = cst.tile([128, HPC], f32)
            nc.sync.dma_start(out=epss_t, in_=epss_in.ap())
            gamcf = cst.tile([128, HPC], f32)
            nc.sync.dma_start(out=gamcf, in_=gamcf_in.ap())
            ones_bc = cst.tile([128, 2 * D_HEAD], f32)
            nc.vector.memset(ones_bc[:], 1.0)

            outw_bc = cst.tile([128, D_MODEL], f32)
            outb_bc = cst.tile([128, D_MODEL], f32)
            nc.sync.dma_start(out=outw_bc, in_=bcast_ap(outw_in))
            nc.sync.dma_start(out=outb_bc, in_=bcast_ap(outb_in))

            # per-rep python state shared between emission closures
            ctx = {}

            def emit_stats_group(rep, g8):
                """LN stats for tiles [g8*GSZ, (g8+1)*GSZ)."""
                t0 = g8 * GSZ
                mv = sp.tile([128, GSZ, 2], f32, tag="mv", name=f"mv{rep}_{g8}")
                for i in range(GSZ):
                    t = t0 + i
                    x_t = xp.tile([128, D_MODEL], bf16, tag="x",
                                  name=f"x{rep}_{t}")
                    nc.sync.dma_start(out=x_t, in_=x_in[t * C:(t + 1) * C, :])
                    stats = sm.tile([128, 2, 6], f32, tag="stats",
                                    name=f"st{rep}_{t}")
                    for i2 in range(2):
                        nc.vector.bn_stats(out=stats[:, i2, :],
                                           in_=x_t[:, i2 * 512:(i2 + 1) * 512])
                    nc.vector.bn_aggr(out=mv[:, i, :], in_=stats[:])
                ln_g = sp.tile([128, GSZ], f32, tag="ln_g", name=f"ln{rep}_{g8}")
                rstd_g = sp.tile([128, GSZ], f32, tag="rstd_g",
                                 name=f"rstd{rep}_{g8}")
                nc.scalar.activation(out=ln_g[:], in_=mv[:, :, 1],
                                     func=Act.Ln, bias=eps_t[:])
                nc.scalar.activation(out=rstd_g[:], in_=ln_g[:],
                                     func=Act.Exp, scale=-0.5)
                msn_bf = sp.tile([128, 2 * GSZ], bf16, tag="msn_bf",
                                 name=f"msn{rep}_{g8}")
                msn_v = msn_bf[:].rearrange("p (t s) -> p t s", s=2)
                nc.vector.tensor_scalar_mul(out=msn_v[:, :, 0:1],
                                            in0=mv[:, :, 0:1], scalar1=-1.0)
                nc.scalar.activation(
                    out=msn_v[:, :, 1:2],
                    in_=ln_g[:].rearrange("p (t o) -> p t o", o=1),
                    func=Act.Exp, scale=0.5)
                # transpose each tile's [-mu, std] column pair to a [2, C]
                # block at base partition 0 (PE transpose + small copy)
                mustd_g = sp.tile([2, GSZ * C], bf16, tag="mustd_g",
                                  name=f"mustd{rep}_{g8}")
                for i in range(GSZ):
                    tpm = ptp.tile([128, 3, 2 * D_HEAD], bf16, tag="tp",
                                   name=f"tpm{rep}_{g8}_{i}")
                    reg = tpm[0:2, 0, (i % 2) * C:(i % 2) * C + C]
                    nc.tensor.matmul(reg, msn_bf[:, 2 * i:2 * i + 2], ident[:],
                                     is_transpose=True)
                    nc.vector.tensor_copy(out=mustd_g[:, i * C:(i + 1) * C],
                                          in_=reg)
                if "mustd" in dbg:
                    nc.sync.dma_start(out=dbg["mustd"][g8], in_=mustd_g[:])
                ctx[("rstd", g8)] = rstd_g
                ctx[("mustd", g8)] = mustd_g

            def emit_stage_a(rep, t):
                g8 = t // GSZ
                rstd = ctx[("rstd", g8)][:, t % GSZ:t % GSZ + 1]
                xT = wk.tile([128, KT, C], bf16, tag="xT", name=f"xT{rep}_{t}")
                nc.sync.dma_start(out=xT, in_=xt_in[t])
                ps_vp = pproj.tile([128, 4 * D_HEAD], f32, tag="pvp",
                                   name=f"pvp{rep}_{t}")
                ps_qk = pproj.tile([128, 4 * D_HEAD], f32, tag="pqk",
                                   name=f"pqk{rep}_{t}")
                for k in range(KT):
                    nc.tensor.matmul(ps_vp[:], xT[:, k, :], wvp_sb[:, k, :],
                                     start=(k == 0), stop=False)
                    nc.tensor.matmul(ps_qk[:], xT[:, k, :], wq_sb[:, k, :],
                                     start=(k == 0), stop=False)
                mustd = ctx[("mustd", g8)][:, (t % GSZ) * C:(t % GSZ + 1) * C]
                nc.tensor.matmul(ps_vp[:], mustd, fvp_sb[:],
                                 start=False, stop=True)
                nc.tensor.matmul(ps_qk[:], mustd, fq_sb[:],
                                 start=False, stop=True)

                # feature maps (Act: Exp only; one act table for the program)
                expq = wk.tile([128, 2 * D_HEAD], bf16, tag="expq",
                               name=f"expq{rep}_{t}")
                zq = sm.tile([128, HPC], f32, tag="zq", name=f"zq{rep}_{t}")
                expk = wk.tile([128, 2 * D_HEAD], f32, tag="expk",
                               name=f"expk{rep}_{t}")
                zk = sm.tile([128, HPC], f32, tag="zk", name=f"zk{rep}_{t}")
                for h in range(HPC):
                    hs = slice(h * D_HEAD, (h + 1) * D_HEAD)
                    ks = slice(2 * D_HEAD + h * D_HEAD,
                               2 * D_HEAD + (h + 1) * D_HEAD)
                    nc.scalar.activation(out=expq[:, hs], in_=ps_qk[:, hs],
                                         func=Act.Exp, scale=rstd,
                                         accum_out=zq[:, h:h + 1])
                    nc.scalar.activation(out=expk[:, hs], in_=ps_qk[:, ks],
                                         func=Act.Exp, scale=rstd,
                                         accum_out=zk[:, h:h + 1])
                # silu(p) = rstd*ps_p / (1 + exp(-rstd*ps_p))
                nrstd = sm.tile([128, 1], f32, tag="nrstd", name=f"nr{rep}_{t}")
                nc.vector.tensor_scalar_mul(out=nrstd[:], in0=rstd, scalar1=-1.0)
                emp = wk.tile([128, 2 * D_HEAD], f32, tag="emp",
                              name=f"emp{rep}_{t}")
                nc.scalar.activation(out=emp[:],
                                     in_=ps_vp[:, 2 * D_HEAD:4 * D_HEAD],
                                     func=Act.Exp, scale=nrstd[:])
                nc.gpsimd.tensor_tensor(out=emp[:], in0=emp[:], in1=ones_bc[:],
                                        op=Alu.add)
                rsig = wk.tile([128, 2 * D_HEAD], f32, tag="rsig",
                               name=f"rsig{rep}_{t}")
                nc.vector.reciprocal(out=rsig[:], in_=emp[:])
                silu_p = wk.tile([128, 2 * D_HEAD], f32, tag="silup",
                                 name=f"silu{rep}_{t}")
                nc.vector.scalar_tensor_tensor(
                    out=silu_p[:], in0=rsig[:], scalar=rstd,
                    in1=ps_vp[:, 2 * D_HEAD:4 * D_HEAD],
                    op0=Alu.mult, op1=Alu.mult)

                # kn = softmax(k) = expk / zk
                rzk = sm.tile([128, HPC], f32, tag="rzk", name=f"rzk{rep}_{t}")
                nc.vector.reciprocal(out=rzk[:], in_=zk[:])
                kn = wk.tile([128, 2 * D_HEAD], bf16, tag="kn",
                             name=f"kn{rep}_{t}")
                for h in range(HPC):
                    hs = slice(h * D_HEAD, (h + 1) * D_HEAD)
                    nc.vector.tensor_scalar_mul(out=kn[:, hs],
                                                in0=expk[:, hs],
                                                scalar1=rzk[:, h:h + 1])

                # v_aug
                v_aug = wk.tile([128, HPC, D_HEAD + 1], bf16, tag="vaug",
                                name=f"va{rep}_{t}")
                nc.vector.tensor_scalar_mul(
                    out=v_aug[:, :, 0:D_HEAD],
                    in0=ps_vp[:, 0:2 * D_HEAD].rearrange(
                        "p (h d) -> p h d", h=HPC),
                    scalar1=rstd)
                nc.vector.memset(v_aug[:, :, D_HEAD:D_HEAD + 1], 1.0)
                if "expq" in dbg:
                    nc.sync.dma_start(out=dbg["expq"][t], in_=expq[:])
                if "kn" in dbg:
                    nc.sync.dma_start(out=dbg["kn"][t], in_=kn[:])
                ctx[("A", t)] = (expq, kn, v_aug, silu_p, zq, rstd)

            def emit_stage_b(rep, t):
                expq, kn, v_aug, silu_p, zq, rstd = ctx.pop(("A", t))
                chunk = t % TPB
                if chunk == 0:
                    S_f = st.tile([128, HPC, D_HEAD + 1], f32, tag="Sf",
                                  name=f"Sfi{rep}_{t}")
                    nc.vector.memset(S_f[:], 0.0)
                    S_bf = st.tile([128, HPC, D_HEAD + 1], bf16, tag="Sbf",
                                   name=f"Sbfi{rep}_{t}")
                    nc.gpsimd.memset(S_bf[:], 0.0)
                    ctx["S"] = (S_f, S_bf)
                S_f, S_bf = ctx["S"]

                # transposes on PE: tp bank regions [128, 3, 256] bf16
                tp = ptp.tile([128, 3, 2 * D_HEAD], bf16, tag="tp",
                              name=f"tp{rep}_{t}")
                for h in range(HPC):
                    hs = slice(h * D_HEAD, (h + 1) * D_HEAD)
                    nc.tensor.matmul(tp[:, 0, hs], expq[:, hs], ident[:],
                                     is_transpose=True)
                    nc.tensor.matmul(tp[:, 1, hs], kn[:, hs], ident[:],
                                     is_transpose=True)
                qT = wk.tile([128, HPC, C], bf16, tag="qT", name=f"qT{rep}_{t}")
                nc.vector.tensor_copy(out=qT[:], in_=tp[:, 0, :].rearrange(
                    "p (h d) -> p h d", h=HPC))
                # knS: [carry | knT] with a leading carry column
                knS = st.tile([128, HPC, C + 1], bf16, tag="knS",
                              name=f"knS{rep}_{t}")
                nc.vector.tensor_copy(out=knS[:, :, 1:C + 1],
                                      in_=tp[:, 1, :].rearrange(
                                          "p (h d) -> p h d", h=HPC))
                if chunk == 0:
                    nc.vector.memset(knS[:, :, 0:1], 0.0)
                else:
                    nc.vector.tensor_copy(out=knS[:, :, 0:1],
                                          in_=ctx["knS_prev"][:, :, C:C + 1])
                ctx["knS_prev"] = knS

                # smear in the transposed domain:
                #   ktT[:,h,j] = omsig_h*knT[:,h,j] + sig_h*knT[:,h,j-1]
                ktT = wk.tile([128, HPC, C], bf16, tag="ktT",
                              name=f"ktT{rep}_{t}")
                nc.gpsimd.tensor_tensor(
                    out=ktT[:], in0=knS[:, :, 1:C + 1],
                    in1=omsigf[:].rearrange("p (h d) -> p h d", h=HPC),
                    op=Alu.mult)
                ksh = wk.tile([128, HPC, C], bf16, tag="ksh",
                              name=f"ksh{rep}_{t}")
                nc.gpsimd.tensor_tensor(
                    out=ksh[:], in0=knS[:, :, 0:C],
                    in1=sigf[:].rearrange("p (h d) -> p h d", h=HPC),
                    op=Alu.mult)
                nc.gpsimd.tensor_tensor(out=ktT[:], in0=ktT[:], in1=ksh[:],
                                        op=Alu.add)

                # attention scores + transpose-back of ktT for the state path
                ato = pato.tile([128, 4, C], f32, tag="atO",
                                name=f"ato{rep}_{t}")
                at_ps = ato[:, 0:2, :]
                o2_ps = ato[:, 2:4, :]
                for h in range(HPC):
                    nc.tensor.matmul(at_ps[:, h, :], ktT[:, h, :], qT[:, h, :],
                                     start=True, stop=True)
                for h in range(HPC):
                    hs = slice(h * D_HEAD, (h + 1) * D_HEAD)
                    nc.tensor.matmul(tp[:, 2, hs], ktT[:, h, :], ident[:],
                                     is_transpose=True)
                atm = wk.tile([128, HPC, C], bf16, tag="atm",
                              name=f"atm{rep}_{t}")
                nc.vector.tensor_tensor(out=atm[:], in0=at_ps[:], in1=dt_sb[:],
                                        op=Alu.mult)
                ktil = wk.tile([128, 2 * D_HEAD], bf16, tag="ktil",
                               name=f"ktil{rep}_{t}")
                nc.vector.tensor_copy(out=ktil[:], in_=tp[:, 2, :])
                kmu = wk.tile([128, 2 * D_HEAD], bf16, tag="kmu",
                              name=f"kmu{rep}_{t}")
                nc.gpsimd.tensor_tensor(out=kmu[:], in0=ktil[:], in1=musf[:],
                                        op=Alu.mult)

                # o1 (intra-chunk, with ones column), o2 (state), o2d (den)
                o1b = pO1.tile([128, 2 * (D_HEAD + 1) + HPC], f32, tag="o1b",
                               name=f"o1b{rep}_{t}")
                o1_ps = o1b[:, 0:2 * (D_HEAD + 1)].rearrange(
                    "p (h d) -> p h d", h=HPC)
                o2d = o1b[:, 2 * (D_HEAD + 1):2 * (D_HEAD + 1) + HPC]
                s_ps4 = pS.tile([128, HPC, D_HEAD + 1], f32, tag="sps",
                                name=f"sps{rep}_{t}")
                for h in range(HPC):
                    hs = slice(h * D_HEAD, (h + 1) * D_HEAD)
                    nc.tensor.matmul(o2_ps[:, h, :], qT[:, h, :],
                                     S_bf[:, h, 0:D_HEAD], start=True, stop=True)
                    nc.tensor.matmul(o2d[:, h:h + 1], qT[:, h, :],
                                     S_bf[:, h, D_HEAD:D_HEAD + 1],
                                     start=True, stop=True)
                    nc.tensor.matmul(o1_ps[:, h, :], atm[:, h, :],
                                     v_aug[:, h, :], start=True, stop=True)
                    nc.tensor.matmul(s_ps4[:, h, :], kmu[:, hs],
                                     v_aug[:, h, :], start=True, stop=True)

                # combine: o = lam*o2 + o1 ; den = eps*s^2*zq + lam*o2d + o1[.,128]
                o_sb = wk.tile([128, HPC, D_HEAD], f32, tag="osb",
                               name=f"osb{rep}_{t}")
                nc.vector.tensor_tensor(out=o_sb[:], in0=o2_ps[:], in1=lamf[:],
                                        op=Alu.mult)
                nc.vector.tensor_tensor(
                    out=o_sb[:], in0=o_sb[:],
                    in1=o1_ps[:, :, 0:D_HEAD], op=Alu.add)
                den = sm.tile([128, HPC], f32, tag="den", name=f"den{rep}_{t}")
                nc.vector.tensor_tensor(out=den[:], in0=zq[:], in1=epss_t[:],
                                        op=Alu.mult)
                d2 = sm.tile([128, HPC], f32, tag="d2", name=f"d2{rep}_{t}")
                nc.vector.tensor_tensor(out=d2[:], in0=o2d[:], in1=lam2[:],
                                        op=Alu.mult)
                nc.vector.tensor_tensor(out=den[:], in0=den[:], in1=d2[:],
                                        op=Alu.add)
                nc.vector.tensor_tensor(
                    out=den[:], in0=den[:],
                    in1=o1_ps[:, :, D_HEAD:D_HEAD + 1].rearrange(
                        "p h o -> p (h o)"),
                    op=Alu.add)
                rden = sm.tile([128, HPC], f32, tag="rden", name=f"rd{rep}_{t}")
                nc.vector.reciprocal(out=rden[:], in_=den[:])
                po = wk.tile([128, 2 * D_HEAD], bf16, tag="po",
                             name=f"po{rep}_{t}")
                for h in range(HPC):
                    hs = slice(h * D_HEAD, (h + 1) * D_HEAD)
                    nc.vector.scalar_tensor_tensor(
                        out=po[:, hs], in0=o_sb[:, h, :],
                        scalar=rden[:, h:h + 1], in1=silu_p[:, hs],
                        op0=Alu.mult, op1=Alu.mult)

                # state update (f32 master + bf16 matmul copy)
                S_f_new = st.tile([128, HPC, D_HEAD + 1], f32, tag="Sf",
                                  name=f"Sf{rep}_{t}")
                for h in range(HPC):
                    nc.vector.scalar_tensor_tensor(
                        out=S_f_new[:, h, :], in0=S_f[:, h, :],
                        scalar=gamcf[:, h:h + 1], in1=s_ps4[:, h, :],
                        op0=Alu.mult, op1=Alu.add)
                S_bf_new = st.tile([128, HPC, D_HEAD + 1], bf16, tag="Sbf",
                                   name=f"Sbf{rep}_{t}")
                nc.gpsimd.tensor_copy(out=S_bf_new[:], in_=S_f_new[:])
                ctx["S"] = (S_f_new, S_bf_new)

                if "ktT" in dbg:
                    nc.sync.dma_start(out=dbg["ktT"][t], in_=ktT[:])
                if "po" in dbg:
                    nc.sync.dma_start(out=dbg["po"][t], in_=po[:])
                if "osb" in dbg:
                    nc.sync.dma_start(
                        out=dbg["osb"][t],
                        in_=o_sb[:].rearrange("p h d -> p (h d)"))
                # ship po row-major
                g, dest = t // GSZ, t % GSZ
                nc.scalar.dma_start(out=pot_dram[rep % nex, g, dest], in_=po[:])
                if dest == GSZ - 1:
                    nc.gpsimd.collective_compute(
                        "AllToAll", Alu.bypass,
                        replica_groups=[list(range(N_CORES))],
                        ins=[pot_dram[rep % nex, g]],
                        outs=[potex_dram[rep % nex, g]])

            def emit_outproj(rep, g):
                pex = potex_dram[rep % nex, g]
                pox = wk.tile([128, NKT, C], bf16, tag="pox",
                              name=f"pox{rep}_{g}")
                for s in range(N_CORES):
                    for ci in range(HPC):
                        nc.scalar.dma_start_transpose(
                            out=pox[:, s * HPC + ci, :],
                            in_=pex[s, :, ci * 128:(ci + 1) * 128])
                z_sb = zrp.tile([128, D_MODEL], f32, tag="zsb",
                                name=f"zsb{rep}_{g}")
                statf = sm.tile([128, 2, 6], f32, tag="statf",
                                name=f"stf{rep}_{g}")
                for half in range(2):
                    ns = slice(half * 512, (half + 1) * 512)
                    z_ps = pproj.tile([128, 512], f32, tag="pvp",
                                      name=f"z{rep}_{g}_{half}")
                    for kt in range(NKT):
                        nc.tensor.matmul(z_ps[:], pox[:, kt, :],
                                         wout_sb[:, kt, ns],
                                         start=(kt == 0), stop=(kt == NKT - 1))
                    nc.vector.tensor_copy(out=z_sb[:, ns], in_=z_ps[:])
                    nc.vector.bn_stats(out=statf[:, half, :], in_=z_sb[:, ns])
                mvf = sm.tile([128, 2], f32, tag="mvf", name=f"mvf{rep}_{g}")
                nc.vector.bn_aggr(out=mvf[:], in_=statf[:])
                lnf = sm.tile([128, 1], f32, tag="lnf", name=f"lnf{rep}_{g}")
                nc.scalar.activation(out=lnf[:], in_=mvf[:, 1:2],
                                     func=Act.Ln, bias=eps_t[:])
                rstdf = sm.tile([128, 1], f32, tag="rstdf", name=f"rsf{rep}_{g}")
                nc.scalar.activation(out=rstdf[:], in_=lnf[:],
                                     func=Act.Exp, scale=-0.5)
                o_t = xp.tile([128, D_MODEL], f32, tag="y", name=f"y{rep}_{g}")
                nc.vector.tensor_scalar(
                    out=o_t[:], in0=z_sb[:], scalar1=mvf[:, 0:1],
                    scalar2=rstdf[:], op0=Alu.subtract, op1=Alu.mult)
                nc.gpsimd.tensor_tensor(out=o_t[:], in0=o_t[:],
                                        in1=outw_bc[:], op=Alu.mult)
                nc.gpsimd.tensor_tensor(out=o_t[:], in0=o_t[:],
                                        in1=outb_bc[:], op=Alu.add)
                nc.sync.dma_start(out=out_ext[g * C:(g + 1) * C, :], in_=o_t[:])

            for rep in range(reps):
                emitted = set()
                emit_stats_group(rep, 0)
                emit_stage_a(rep, 0)
                for t in range(NT):
                    if t % GSZ == 0 and (t // GSZ) + 1 < GROUPS:
                        emit_stats_group(rep, t // GSZ + 1)
                    if t + 1 < NT:
                        emit_stage_a(rep, t + 1)
                    emit_stage_b(rep, t)
                    if t >= GSZ - 1 + OUTPROJ_DELAY and \
                            (t - OUTPROJ_DELAY) % GSZ == GSZ - 1:
                        gd = (t - OUTPROJ_DELAY) // GSZ
                        emit_outproj(rep, gd)
                        emitted.add(gd)
                for g in range(GROUPS):
                    if g not in emitted:
                        emit_outproj(rep, g)

    nc.compile()
    return nc


def prepare_in_maps(inputs: dict):
    """Host-side: fold LN affines into weights, slice per core, build
    per-head decay/smear constants, cast to bf16."""
    import ml_dtypes
    bf = ml_dtypes.bfloat16

    x = np.ascontiguousarray(np.asarray(inputs["x"], np.float32)
                             .reshape(ROWS, D_MODEL)).astype(bf)
    # [t, p, kt*128+r] = x[t*128+r, kt*128+p]
    xt = np.ascontiguousarray(
        x.reshape(NT, C, KT, 128).transpose(0, 3, 2, 1).reshape(
            NT, 128, D_MODEL))
    W_in = np.asarray(inputs["W_in"], np.float32)
    W_out = np.asarray(inputs["W_out"], np.float32)
    Wq = np.asarray(inputs["Wq"], np.float32)
    Wk = np.asarray(inputs["Wk"], np.float32)
    bq = np.asarray(inputs["bq"], np.float32)
    bk = np.asarray(inputs["bk"], np.float32)
    in_w = np.asarray(inputs["in_ln_w"], np.float32)
    in_b = np.asarray(inputs["in_ln_b"], np.float32)
    q_w = np.asarray(inputs["q_ln_w"], np.float32)
    q_b = np.asarray(inputs["q_ln_b"], np.float32)
    k_w = np.asarray(inputs["k_ln_w"], np.float32)
    k_b = np.asarray(inputs["k_ln_b"], np.float32)
    outw = np.asarray(inputs["out_ln_w"], np.float32)
    outb = np.asarray(inputs["out_ln_b"], np.float32)
    smear = np.asarray(inputs["smear_factor"], np.float32)
    log_scale = np.asarray(inputs["log_scale"], np.float32)

    Wvp_f = W_in * in_w[:, None]
    bvp_f = in_b @ W_in
    Wq_f = Wq * q_w[:, None]
    bq_f = bq + q_b @ Wq
    Wk_f = Wk * k_w[:, None]
    bk_f = bk + k_b @ Wk

    sigm = 1.0 / (1.0 + np.exp(-smear.astype(np.float64)))
    s = np.exp(log_scale.astype(np.float64))

    a = np.arange(C)
    diff = a[:, None] - a[None, :]          # i - j
    in_maps = []
    for c in range(N_CORES):
        heads = [HPC * c + i for i in range(HPC)]
        vcols = np.concatenate(
            [np.arange(h * D_HEAD, (h + 1) * D_HEAD) for h in heads])
        pcols = vcols + D_EXP
        dts, lams, lam2s, muss, sigs, omsigs, epsss, gamcs = \
            [], [], [], [], [], [], [], []
        for h in heads:
            lg = -_SLOPES[h]                 # log gamma
            D = np.where(diff >= 0, np.exp(lg * diff), 0.0)   # [i, j]
            dts.append(D.T.astype(np.float32))                # [j, i]
            lam = np.exp(lg * (a + 1))
            lams.append(np.repeat(lam[:, None], D_HEAD, 1).astype(np.float32))
            lam2s.append(lam.astype(np.float32))
            muss.append(np.repeat(
                np.exp(lg * (C - 1 - a))[:, None], D_HEAD, 1).astype(np.float32))
            sigs.append(np.full((C, D_HEAD), sigm[h], np.float32))
            omsigs.append(np.full((C, D_HEAD), 1.0 - sigm[h], np.float32))
            epsss.append(np.full(C, ATTN_EPS * s[h] * s[h], np.float32))
            gamcs.append(np.full(C, np.exp(lg * C), np.float32))
        wvp_c = np.ascontiguousarray(
            np.concatenate([Wvp_f[:, vcols], Wvp_f[:, pcols]], axis=1))
        bvp_c = np.concatenate([bvp_f[vcols], bvp_f[pcols]])
        wq_c = Wq_f[:, vcols]
        wk_c = Wk_f[:, vcols]
        in_maps.append({
            "x": x,
            "xt": xt,
            "wvp": wvp_c.astype(bf),
            "fvp": np.ascontiguousarray(
                np.stack([wvp_c.sum(0), bvp_c])).astype(bf),
            "wq": np.ascontiguousarray(
                np.concatenate([wq_c, wk_c], axis=1)).astype(bf),
            "fq": np.ascontiguousarray(np.stack([
                np.concatenate([wq_c.sum(0), wk_c.sum(0)]),
                np.concatenate([bq_f[vcols], bk_f[vcols]])])).astype(bf),
            "wout": W_out.astype(bf),
            "outw": outw, "outb": outb,
            "dtmask": np.stack(dts),
            "lamf": np.ascontiguousarray(np.stack(lams, axis=1)),  # [C,HPC,128]
            "lam2": np.ascontiguousarray(np.stack(lam2s, axis=1)),  # [C,HPC]
            "musf": np.ascontiguousarray(
                np.concatenate(muss, axis=1)).astype(bf),     # [C, 256]
            "sigf": np.ascontiguousarray(
                np.concatenate(sigs, axis=1)).astype(bf),
            "omsigf": np.ascontiguousarray(
                np.concatenate(omsigs, axis=1)).astype(bf),
            "epss": np.ascontiguousarray(np.stack(epsss, axis=1)),  # [C, HPC]
            "gamcf": np.ascontiguousarray(np.stack(gamcs, axis=1)),
        })
    return in_maps


def assemble_output(outs: list) -> np.ndarray:
    """outs[c] is [512, 1024]: groups g=0..3 rows [g*128:(g+1)*128] hold
    global row tile (g*8 + c)."""
    full = np.empty((ROWS, D_MODEL), np.float32)
    for ti in range(NT):
        c, g = ti % GSZ, ti // GSZ
        full[ti * C:(ti + 1) * C] = outs[c][g * C:(g + 1) * C]
    return full.reshape(B, L, D_MODEL)


DEFAULT_MM_DT = "bf16"

_CACHED = {}


def _get_runner(mm_dt=None, reps=1):
    key = (mm_dt, reps)
    if key not in _CACHED:
        nc = build_kernel(reps=reps)
        _CACHED[key] = nc
    return _CACHED[key]


def kernel(**inputs) -> np.ndarray:
    nc = _get_runner()
    in_maps = prepare_in_maps(inputs)
    from concourse.bass_utils import run_bass_kernel_spmd
    res = run_bass_kernel_spmd(nc, in_maps, list(range(N_CORES)))
    return assemble_output([res.results[c]["out"] for c in range(N_CORES)])
